# revision 4
# baseline (speedup 1.0000x reference)
"""BiGRU encoder (2-layer, bidirectional) Trainium2 Bass kernel.

Strategy (per core, batch-parallel over N=64 -> B=8 per core):
  P0: layer-0 input projections gx = W_ih @ x^T + bias (transposed layout).
  P1: layer-0 recurrence, fwd+bwd chains interleaved on one core.
      Transposed state h^T [128p x (2 chunks, B)]; W_hh stationary blocks;
      gx_rz + b_hh_n folded into PSUM via identity-matmuls; gates on ACT/DVE.
  P2: layer-1 projections from [f0; b0].
  P3: layer-1 recurrence -> fp16 outputs.

The wall-clock here is dominated by the axon host<->device tunnel
(~40-90 MB/s), so the run path minimizes bytes moved:
  - inputs cast to fp16 on host before upload;
  - outputs come back as fp16 (cast to fp32 on host);
  - donated output buffers are created on-device (no zeros upload);
  - the jitted executable is built once and cached across calls;
  - weight upload is dispatched async while the host preps x.
"""

import os
import sys

sys.path.insert(0, "/opt/trn_rl_repo")

import numpy as np

import concourse.bacc as bacc
import concourse.bass as bass
import concourse.tile as tile
from concourse import mybir
from concourse.bass_utils import run_bass_kernel_spmd

T, N, D_IN, H = 2000, 64, 512, 256
NCORES = 8
B = N // NCORES          # batch per core
G3 = 6                   # 3H / 128 output chunks
HC = 2                   # H / 128 state chunks
KC = 4                   # input-feature chunks (512/128), same for l0 and l1

MODE = os.environ.get("GRU_MODE", "fp16")  # "fp32" | "fp16"

F32 = mybir.dt.float32
AF = mybir.ActivationFunctionType
OP = mybir.AluOpType


def _wd(mode):
    return F32 if mode == "fp32" else mybir.dt.float16


def _wd_np(mode):
    return np.float32 if mode == "fp32" else np.float16


def build_program(t=T, blk=100, p_steps=50, mode=MODE, b=B,
                  no_imm2=False, fp16_state=False, stag=False,
                  gp_blend=False, psum_bufs=2, sp_bufs=3, npre_psum=False,
                  a1_split=False):
    """Build the full 4-phase program. t must be divisible by blk and p_steps."""
    assert t % blk == 0 and t % p_steps == 0
    nblk = t // blk
    np_tiles = t // p_steps
    WD = _wd(mode)

    nc = bacc.Bacc("TRN2", target_bir_lowering=False, debug=False,
                   num_devices=NCORES)

    dirs = ("f", "b")
    # ---- DRAM I/O ----
    xT = nc.dram_tensor("xT", [KC, 128, t, b], WD, kind="ExternalInput").ap()
    ident = nc.dram_tensor("ident", [128, 128], WD, kind="ExternalInput").ap()
    wih, whh, biasd, bhn = {}, {}, {}, {}
    for l in (0, 1):
        for d in dirs:
            k = f"{l}{d}"
            wih[k] = nc.dram_tensor(f"wih_{k}", [KC, G3, 128, 128], WD,
                                    kind="ExternalInput").ap()
            whh[k] = nc.dram_tensor(f"whh_{k}", [HC, G3, 128, 128], WD,
                                    kind="ExternalInput").ap()
            biasd[k] = nc.dram_tensor(f"bias_{k}", [128, G3], F32,
                                      kind="ExternalInput").ap()
            bhn[k] = nc.dram_tensor(f"bhn_{k}", [128, HC, b], WD,
                                    kind="ExternalInput").ap()
    gxrz, gxn = {}, {}
    for k in ("0f", "0b", "1f", "1b"):
        gxrz[k] = nc.dram_tensor(f"gxrz_{k}", [4, 128, t, b], WD).ap()
        gxn[k] = nc.dram_tensor(f"gxn_{k}", [2, 128, t, b], F32).ap()
    hh = {d: nc.dram_tensor(f"hh0{d}", [HC, 128, t, b], WD).ap() for d in dirs}
    out = {d: nc.dram_tensor(f"out1{d}", [HC, 128, t, b], WD,
                             kind="ExternalOutput").ap() for d in dirs}

    opts = dict(no_imm2=no_imm2, fp16_state=fp16_state, stag=stag,
                gp_blend=gp_blend, psum_bufs=psum_bufs, sp_bufs=sp_bufs,
                npre_psum=npre_psum, a1_split=a1_split)
    with tile.TileContext(nc) as tc:
        _emit(tc, nc, mode, t, blk, nblk, p_steps, np_tiles, b,
              xT, ident, wih, whh, biasd, bhn, gxrz, gxn, hh, out, opts)

    nc.compile()
    return nc


def _emit(tc, nc, mode, t, blk, nblk, p_steps, np_tiles, b,
          xT, ident, wih, whh, biasd, bhn, gxrz, gxn, hh, out, opts):
    from contextlib import ExitStack
    ctx = ExitStack()
    WD = _wd(mode)
    dirs = ("f", "b")
    fp16 = mode != "fp32"

    # ---- persistent SBUF: weights, identity, biases ----
    wpool = ctx.enter_context(tc.tile_pool(name="weights", bufs=1))
    wih_sb, whh_sb, bias_sb, bhn_sb = {}, {}, {}, {}
    for l in (0, 1):
        for d in dirs:
            k = f"{l}{d}"
            wih_sb[k] = wpool.tile([128, KC, G3, 128], WD, name=f"wihsb_{k}")
            nc.sync.dma_start(wih_sb[k][:],
                              wih[k].rearrange("k m p q -> p k m q"))
            whh_sb[k] = wpool.tile([128, HC, G3, 128], WD, name=f"whhsb_{k}")
            nc.sync.dma_start(whh_sb[k][:],
                              whh[k].rearrange("k m p q -> p k m q"))
            bias_sb[k] = wpool.tile([128, G3], F32, name=f"biassb_{k}")
            nc.sync.dma_start(bias_sb[k][:], biasd[k])
            bhn_sb[k] = wpool.tile([128, HC, b], WD, name=f"bhnsb_{k}")
            nc.sync.dma_start(bhn_sb[k][:], bhn[k])
    id_sb = wpool.tile([128, 128], WD, name="id_sb")
    nc.sync.dma_start(id_sb[:], ident)

    loop_kw = (dict(staggered_reset=True,
                    hint_engines=(mybir.EngineType.PE,))
               if opts.get('stag') else {})

    # ================= projections =================
    def projection(layer, rhs_load):
        """rhs_load(iv, xsb) emits DMAs filling xsb [128, KC, p_steps, b]."""
        cols = p_steps * b
        with tc.tile_pool(name=f"pj{layer}", bufs=2) as pool, \
             tc.tile_pool(name=f"pjp{layer}", bufs=3, space="PSUM") as pp:
            def body(iv):
                for d in dirs:
                    k = f"{layer}{d}"
                    xsb = pool.tile([128, KC, p_steps, b], WD, name=f"xsb{k}",
                                    tag="xsb")
                    rhs_load(iv, xsb)
                    for m in range(G3):
                        ps = pp.tile([128, cols], F32, name=f"ps{k}", tag="ps")
                        for kk in range(KC):
                            nc.tensor.matmul(
                                ps[:], wih_sb[k][:, kk, m, :],
                                xsb[:, kk, :, :],
                                start=(kk == 0), stop=(kk == KC - 1))
                        if m < 4:
                            ev = pool.tile([128, cols], WD, name=f"ev{k}",
                                           tag="ev16")
                            dst = gxrz[k][m, :, :, :]
                        else:
                            ev = pool.tile([128, cols], F32, name=f"evn{k}",
                                           tag="ev32")
                            dst = gxn[k][m - 4, :, :, :]
                        nc.scalar.activation(ev[:], ps[:], AF.Identity,
                                             bias=bias_sb[k][:, m:m + 1])
                        nc.sync.dma_start(
                            dst[:, bass.ds(iv * p_steps, p_steps), :],
                            ev[:].rearrange("p (s b) -> p s b", b=b))
            if np_tiles % 2 == 0:
                with tc.For_i(0, np_tiles // 2, 1, **loop_kw) as iv:
                    body(iv * 2)
                    body(iv * 2 + 1)
            else:
                with tc.For_i(0, np_tiles, 1, **loop_kw) as iv:
                    body(iv)

    def load_x(iv, xsb):
        nc.sync.dma_start(
            xsb[:],
            xT[:, :, bass.ds(iv * p_steps, p_steps), :]
            .rearrange("k p s b -> p k s b"))

    def load_h01(iv, xsb):
        nc.sync.dma_start(
            xsb[:, 0:HC, :, :],
            hh["f"][:, :, bass.ds(iv * p_steps, p_steps), :]
            .rearrange("k p s b -> p k s b"))
        nc.sync.dma_start(
            xsb[:, HC:2 * HC, :, :],
            hh["b"][:, :, bass.ds(iv * p_steps, p_steps), :]
            .rearrange("k p s b -> p k s b"))

    # ================= recurrence =================
    def recurrence(layer, hist_out_dram, f32_state):
        """hist_out_dram: {d: dram ap [HC,128,t,b]} target for history (dtype
        WD). f32_state: keep an fp32 copy of the recurrent state for accuracy
        (the DMA'd history is always the fp16 h16 tile)."""
        rp = ctx.enter_context(tc.tile_pool(name=f"rec{layer}", bufs=1))
        hbW = {d: rp.tile([128, HC, b], WD, name=f"hbW{layer}{d}")
               for d in dirs}
        hb32 = {d: rp.tile([128, HC, b], F32, name=f"hb32{layer}{d}")
                for d in dirs} if (fp16 and f32_state) else hbW
        for d in dirs:
            nc.gpsimd.memset(hbW[d][:], 0.0)
            if fp16 and f32_state:
                nc.gpsimd.memset(hb32[d][:], 0.0)

        with tc.tile_pool(name=f"rgx{layer}", bufs=2) as gp, \
             tc.tile_pool(name=f"rh{layer}", bufs=2) as hp, \
             tc.tile_pool(name=f"rg{layer}", bufs=opts["sp_bufs"]) as sp, \
             tc.tile_pool(name=f"rps{layer}", bufs=opts["psum_bufs"],
                          space="PSUM") as pp:
            def blk_body(iv):
                tiles = {}
                for d in dirs:
                    k = f"{layer}{d}"
                    if d == "f":
                        t0 = iv * blk
                    else:
                        t0 = (nblk - 1) * blk - iv * blk
                    grz = gp.tile([128, 4, blk, b], WD, name=f"grz{k}",
                                  tag="grz")
                    nc.sync.dma_start(
                        grz[:], gxrz[k][:, :, bass.ds(t0, blk), :]
                        .rearrange("k p s b -> p k s b"))
                    gn = gp.tile([128, 2, blk, b], F32, name=f"gn{k}",
                                 tag="gn")
                    nc.sync.dma_start(
                        gn[:], gxn[k][:, :, bass.ds(t0, blk), :]
                        .rearrange("k p s b -> p k s b"))
                    h16 = hp.tile([128, HC, blk, b], WD, name=f"h16{k}",
                                  tag="h16")
                    h32 = (hp.tile([128, HC, blk, b], F32, name=f"h32{k}",
                                   tag="h32")
                           if (fp16 and f32_state) else h16)
                    tiles[d] = (t0, grz, gn, h16, h32)

                for j in range(blk):
                    for d in dirs:
                        k = f"{layer}{d}"
                        t0, grz, gn, h16, h32 = tiles[d]
                        jx = j if d == "f" else blk - 1 - j
                        jp = (j - 1) if d == "f" else (blk - j)
                        no_imm2 = opts.get('no_imm2')
                        st16 = not (fp16 and f32_state)
                        psrz = pp.tile([128, 4, b], F32, name=f"psrz{k}",
                                       tag="psrz")
                        psn = pp.tile([128, 2, b], F32, name=f"psn{k}",
                                      tag="psn")
                        nc.tensor.matmul(psrz[:], id_sb[:],
                                         grz[:, :, jx, :],
                                         start=True, stop=False)
                        if not no_imm2:
                            nc.tensor.matmul(psn[:], id_sb[:],
                                             bhn_sb[k][:],
                                             start=True, stop=False)
                        hprev = (h16[:, :, jp, :] if j > 0 else hbW[d][:])
                        hprev32 = ((h32[:, :, jp, :] if j > 0 else hb32[d][:])
                                   if (fp16 and not st16) else hprev)
                        for m in range(G3):
                            tgt = psrz[:, m, :] if m < 4 else psn[:, m - 4, :]
                            last = (m == 3) if m < 4 else (m == G3 - 1)
                            for kk in range(HC):
                                nc.tensor.matmul(
                                    tgt,
                                    whh_sb[k][:, kk, m, :],
                                    hprev[:, kk, :],
                                    start=(no_imm2 and m == 4 and kk == 0),
                                    stop=(last and kk == HC - 1))
                        rz = sp.tile([128, 4, b], F32, name=f"rz{k}", tag="rz")
                        if opts.get('a1_split'):
                            nc.scalar.activation(rz[:, 0:2, :],
                                                 psrz[:, 0:2, :], AF.Sigmoid)
                            nc.scalar.activation(rz[:, 2:4, :],
                                                 psrz[:, 2:4, :], AF.Sigmoid)
                        else:
                            nc.scalar.activation(rz[:], psrz[:], AF.Sigmoid)
                        rhn = sp.tile([128, 2, b], F32, name=f"rhn{k}",
                                      tag="rhn")
                        if no_imm2:
                            for kk in range(HC):
                                nc.vector.scalar_tensor_tensor(
                                    rhn[:, kk, :], psn[:, kk, :],
                                    bhn_sb[k][:, kk, 0:1], rz[:, kk, :],
                                    op0=OP.add, op1=OP.mult)
                        else:
                            nc.vector.tensor_tensor(rhn[:], rz[:, 0:2, :],
                                                    psn[:], op=OP.mult)
                        if opts.get('npre_psum'):
                            npre = pp.tile([128, 2, b], F32, name=f"npp{k}",
                                           tag="npp")
                        else:
                            npre = sp.tile([128, 2, b], F32, name=f"npre{k}",
                                           tag="npre")
                        nc.vector.tensor_tensor(npre[:], rhn[:],
                                                gn[:, :, jx, :], op=OP.add)
                        nt = sp.tile([128, 2, b], F32, name=f"nt{k}", tag="nt")
                        nc.scalar.activation(nt[:], npre[:], AF.Tanh)
                        eng = nc.gpsimd if opts.get('gp_blend') else nc.vector
                        e = sp.tile([128, 2, b], F32, name=f"e{k}", tag="e")
                        eng.tensor_tensor(e[:], hprev32, nt[:],
                                          op=OP.subtract)
                        zd = sp.tile([128, 2, b], F32, name=f"zd{k}", tag="zd")
                        eng.tensor_tensor(zd[:], rz[:, 2:4, :], e[:],
                                          op=OP.mult)
                        if fp16 and not st16:
                            nc.vector.tensor_tensor(h32[:, :, jx, :], nt[:],
                                                    zd[:], op=OP.add)
                            nc.vector.tensor_tensor(h16[:, :, jx, :], nt[:],
                                                    zd[:], op=OP.add)
                        else:
                            nc.vector.tensor_tensor(h16[:, :, jx, :], nt[:],
                                                    zd[:], op=OP.add)

                for d in dirs:
                    k = f"{layer}{d}"
                    t0, grz, gn, h16, h32 = tiles[d]
                    jl = blk - 1 if d == "f" else 0
                    nc.gpsimd.tensor_copy(hbW[d][:], h16[:, :, jl, :])
                    if fp16 and f32_state:
                        nc.gpsimd.tensor_copy(hb32[d][:], h32[:, :, jl, :])
                    nc.sync.dma_start(
                        hist_out_dram[d][:, :, bass.ds(t0, blk), :]
                        .rearrange("k p s b -> p k s b"), h16[:])

            ur = 1
            for cand in (4, 2):
                if nblk % cand == 0:
                    ur = cand
                    break
            with tc.For_i(0, nblk // ur, 1, **loop_kw) as iv:
                for u in range(ur):
                    blk_body(iv * ur + u)

    projection(0, load_x)
    recurrence(0, hh, f32_state=False)
    projection(1, load_h01)
    recurrence(1, out, f32_state=True)
    ctx.close()


# ================= host side =================

def _prep_weights(inputs, mode, b=B):
    """Per-core weight arrays (shared across cores), keyed by dram name."""
    WDn = _wd_np(mode)
    m = {"ident": np.eye(128, dtype=WDn)}
    for l in (0, 1):
        for d, sfx in (("f", ""), ("b", "_r")):
            k = f"{l}{d}"
            w_ih = np.asarray(inputs[f"w_ih_l{l}{sfx}"])   # [768, 512]
            w_hh = np.asarray(inputs[f"w_hh_l{l}{sfx}"])   # [768, 256]
            b_ih = np.asarray(inputs[f"b_ih_l{l}{sfx}"])
            b_hh = np.asarray(inputs[f"b_hh_l{l}{sfx}"])
            m[f"wih_{k}"] = np.ascontiguousarray(
                w_ih.reshape(G3, 128, KC, 128).transpose(2, 0, 3, 1)
            ).astype(WDn)
            m[f"whh_{k}"] = np.ascontiguousarray(
                w_hh.reshape(G3, 128, HC, 128).transpose(2, 0, 3, 1)
            ).astype(WDn)
            bias = (b_ih + b_hh).astype(np.float32).copy()
            bias[512:] = b_ih[512:]
            m[f"bias_{k}"] = np.ascontiguousarray(
                bias.reshape(G3, 128).T).astype(np.float32)
            m[f"bhn_{k}"] = np.ascontiguousarray(
                np.broadcast_to(b_hh[512:].reshape(HC, 128).T[:, :, None],
                                (128, HC, b))).astype(WDn)
    return m


def _prep_x_global(inputs, mode, t=T, b=B):
    """Global concatenated xT: [NCORES*KC, 128, t, b]."""
    WDn = _wd_np(mode)
    x = np.asarray(inputs["inputs"])[:t]                  # [t, N, 512]
    x16 = x.astype(WDn)
    xg = (x16.reshape(t, NCORES, b, KC, 128)
          .transpose(1, 3, 4, 0, 2)
          .reshape(NCORES * KC, 128, t, b))
    return np.ascontiguousarray(xg)


class _Exec:
    """Cached jitted SPMD executor for a compiled Bass program.

    Mirrors bass2jax.run_bass_via_pjrt but (a) is built once and reused,
    (b) creates donated output buffers on-device instead of uploading
    host zeros, and (c) accepts pre-sharded device arrays for overlap.
    """

    def __init__(self, nc, n_cores=NCORES):
        import jax
        import jax.numpy as jnp
        from jax.experimental.shard_map import shard_map
        from jax.sharding import Mesh, NamedSharding, PartitionSpec
        from concourse import bass2jax

        bass2jax.install_neuronx_cc_hook()
        self.jax = jax
        self.nc = nc
        assert nc.dbg_addr is None
        partition_name = (nc.partition_id_tensor.name
                          if nc.partition_id_tensor else None)

        in_names, out_names, out_avals = [], [], []
        for alloc in nc.m.functions[0].allocations:
            if not isinstance(alloc, mybir.MemoryLocationSet):
                continue
            name = alloc.memorylocations[0].name
            if alloc.kind == "ExternalInput":
                if name != partition_name:
                    in_names.append(name)
            elif alloc.kind == "ExternalOutput":
                assert alloc.tensor_shape is not None
                out_names.append(name)
                out_avals.append(jax.core.ShapedArray(
                    tuple(alloc.tensor_shape), mybir.dt.np(alloc.dtype)))
        self.in_names = in_names
        self.out_names = out_names
        n_params, n_outs = len(in_names), len(out_names)
        all_in = list(in_names) + list(out_names)
        if partition_name is not None:
            all_in.append(partition_name)
        all_in = tuple(all_in)

        def _body(*args):
            operands = list(args)
            if partition_name is not None:
                operands.append(bass2jax.partition_id_tensor())
            outs = bass2jax._bass_exec_p.bind(
                *operands,
                out_avals=tuple(out_avals),
                in_names=all_in,
                out_names=tuple(out_names),
                lowering_input_output_aliases=(),
                sim_require_finite=True,
                sim_require_nnan=True,
                nc=nc,
            )
            return tuple(outs)

        devices = jax.devices()[:n_cores]
        assert len(devices) == n_cores
        self.mesh = Mesh(np.asarray(devices), ("core",))
        self.pspec = PartitionSpec("core")
        self.sharding = NamedSharding(self.mesh, self.pspec)
        in_specs = (self.pspec,) * (n_params + n_outs)
        out_specs = (self.pspec,) * n_outs
        donate = tuple(range(n_params, n_params + n_outs))
        self.fn = jax.jit(
            shard_map(_body, mesh=self.mesh, in_specs=in_specs,
                      out_specs=out_specs, check_rep=False),
            donate_argnums=donate, keep_unused=True)

        gshapes = [(n_cores * a.shape[0], *a.shape[1:]) for a in out_avals]
        gdtypes = [a.dtype for a in out_avals]
        self.zeros_fn = jax.jit(
            lambda: tuple(jnp.zeros(s, d) for s, d in zip(gshapes, gdtypes)),
            out_shardings=(self.sharding,) * n_outs)

    def put(self, arr):
        """Async upload of a global (n_cores*dim0, ...) array."""
        return self.jax.device_put(arr, self.sharding)

    def __call__(self, in_map):
        zs = self.zeros_fn()
        args = [in_map[n] for n in self.in_names]
        outs = self.fn(*args, *zs)
        return dict(zip(self.out_names, outs))


_CACHE = {}


def _get_exec(mode, t=T, blk=100, p_steps=50):
    key = (mode, t, blk, p_steps)
    if key not in _CACHE:
        nc = build_program(t=t, blk=blk, p_steps=p_steps, mode=mode,
                           stag=True)
        _CACHE[key] = _Exec(nc)
    return _CACHE[key]


def kernel(**inputs):
    return run(inputs)["out"]


def run(inputs, mode=MODE, t=T, blk=100, p_steps=50, debug=False):
    import time
    tick = time.time()
    times = {}

    ex = _get_exec(mode, t=t, blk=blk, p_steps=p_steps)
    times["build"] = time.time() - tick; tick = time.time()

    # Dispatch weight uploads async, then overlap x prep on the host.
    wm = _prep_weights(inputs, mode)
    times["prep_w"] = time.time() - tick; tick = time.time()
    in_map = {}
    for name, arr in wm.items():
        g = np.broadcast_to(arr[None], (NCORES, *arr.shape)).reshape(
            NCORES * arr.shape[0], *arr.shape[1:])
        in_map[name] = ex.put(np.ascontiguousarray(g))
    times["put_w"] = time.time() - tick; tick = time.time()

    in_map["xT"] = _prep_x_global(inputs, mode, t=t)
    times["prep_x"] = time.time() - tick; tick = time.time()

    res = ex(in_map)
    f = np.asarray(res["out1f"])   # [NCORES*HC, 128, t, B] (fp16)
    bb = np.asarray(res["out1b"])
    times["exec"] = time.time() - tick; tick = time.time()

    outs = np.empty((t, N, 2 * H), dtype=np.float32)
    ov = outs.reshape(t, NCORES, B, 2, HC, 128)
    ov[:, :, :, 0] = f.reshape(NCORES, HC, 128, t, B).transpose(3, 0, 4, 1, 2)
    ov[:, :, :, 1] = bb.reshape(NCORES, HC, 128, t, B).transpose(3, 0, 4, 1, 2)
    times["asm"] = time.time() - tick

    if debug or os.environ.get("GRU_DEBUG"):
        print("  " + "  ".join(f"{k}={v:.2f}s" for k, v in times.items()))
    return {"out": outs, "exec_ns": None, "times": times}
